# revision 25
# baseline (speedup 1.0000x reference)
"""Absolute sinusoidal positional encoding: out = x + pe[None, :, :].

x: [8, 4096, 1024] f32.  pe[s, 2j] = sin(s / 10000^(2j/D)), pe[s, 2j+1] = cos(...).

Sharding: along sequence across 8 cores; core k handles x[:, k*512:(k+1)*512, :].
Per-core kernel is a streaming DVE add over 16 MiB in + 16 MiB out -- pure
HBM-bandwidth bound. The pe table slice is generated on-chip (f32 angles =
s * inv_freq reproduced bit-exactly on DVE, Cody-Waite range reduction, ACT
Sin) from a 2 KiB inv_freq vector + per-core row indices, so no pe bytes
cross HBM. Measured ~95 us/core vs the ~88 us port-bandwidth floor.
"""

import numpy as np

import concourse.tile as tile
from concourse import bacc, mybir
from concourse.bass_utils import run_bass_kernel_spmd

B, S, D = 8, 4096, 1024
N_CORES = 8
S_SH = S // N_CORES          # 512 sequence rows per core
ROWS = B * S_SH              # 4096 flat rows per core
P = 128
NBLK = ROWS // P             # 32 row-blocks of 128
PE_BLK = S_SH // P           # 4 pe row-blocks

# row-blocks per tile (tile bytes = K * 512 KiB); last tiles can be smaller
import os
K = int(os.environ.get("KERN_K", "2"))
ALT_RINGS = os.environ.get("KERN_ALT", "0") == "1"
ONCHIP_PE = os.environ.get("KERN_PE", "onchip") == "onchip"
X_BUFS = int(os.environ.get("KERN_XBUFS", "0"))  # 0 -> one slot per tile
_F32 = mybir.dt.float32
_AL = mybir.AluOpType
_FT = mybir.ActivationFunctionType
_nc_cache = None

HALF = D // 2
_INV2PI = float(np.float32(1.0 / (2 * np.pi)))
_MAGIC = float(np.float32(2.0 ** 23))
_C1 = float(np.float32(402.0 / 64.0))              # 6.28125 (11-bit mantissa)
_C2 = float(np.float32(2 * np.pi - 402.0 / 64.0))  # 2*pi - C1
_HALFPI = float(np.float32(np.pi / 2))


def _emit_pe_block(nc, pool, pe_t, f_t, s_t, m):
    """pe_t[:, m, 0::2] = sin(a), pe_t[:, m, 1::2] = cos(a), a = fl(s*inv_freq).

    The f32 angles match the reference's jnp pos*inv_freq product bit-exactly;
    Cody-Waite reduction (2*pi = C1+C2, k*C1 exact since k<2^10, C1 11-bit)
    keeps the reduced argument within ~1e-7 of the exact a mod 2*pi, and the
    ACT Sin table is accurate on [-pi, pi]. Net pe error ~5e-7 absolute.
    """
    scl = s_t[:, m:m + 1]
    ang = pool.tile([P, HALF], _F32, name=f"ang", tag="ang")
    nc.vector.tensor_scalar(ang[:], f_t[:], scl, None, _AL.mult)
    # sin: r = a - round(a/2pi)*2pi
    tp = pool.tile([P, HALF], _F32, name=f"tp", tag="tp")
    nc.vector.tensor_scalar(tp[:], ang[:], _INV2PI, _MAGIC, _AL.mult, _AL.add)
    k = pool.tile([P, HALF], _F32, name=f"kk", tag="kk")
    nc.vector.tensor_scalar(k[:], tp[:], _MAGIC, None, _AL.subtract)
    m1 = pool.tile([P, HALF], _F32, name=f"m1", tag="m1")
    nc.vector.scalar_tensor_tensor(m1[:], k[:], -_C1, ang[:], _AL.mult, _AL.add)
    r = pool.tile([P, HALF], _F32, name=f"rr", tag="rr")
    nc.vector.scalar_tensor_tensor(r[:], k[:], -_C2, m1[:], _AL.mult, _AL.add)
    nc.scalar.activation(pe_t[:, m, 0:D:2], r[:], _FT.Sin)
    # cos(a) = sin(a + pi/2): re-reduce with quarter-turn offset
    tq = pool.tile([P, HALF], _F32, name=f"tq", tag="tq")
    nc.vector.tensor_scalar(tq[:], ang[:], _INV2PI, 0.25, _AL.mult, _AL.add)
    k2 = pool.tile([P, HALF], _F32, name=f"k2", tag="k2")
    nc.vector.tensor_scalar(k2[:], tq[:], _MAGIC, _MAGIC, _AL.add, _AL.subtract)
    m2 = pool.tile([P, HALF], _F32, name=f"m2", tag="m2")
    nc.vector.scalar_tensor_tensor(m2[:], k2[:], -_C1, ang[:], _AL.mult, _AL.add)
    r2a = pool.tile([P, HALF], _F32, name=f"r2a", tag="r2a")
    nc.vector.scalar_tensor_tensor(r2a[:], k2[:], -_C2, m2[:], _AL.mult, _AL.add)
    r2 = pool.tile([P, HALF], _F32, name=f"r2", tag="r2")
    nc.vector.tensor_scalar(r2[:], r2a[:], _HALFPI, None, _AL.add)
    nc.scalar.activation(pe_t[:, m, 1:D:2], r2[:], _FT.Sin)


def _build_nc():
    global _nc_cache
    if _nc_cache is not None:
        return _nc_cache
    # Bacc (not raw Bass): its finalize runs generate_event_semaphores,
    # which splits multi-sem waits to satisfy the TRN2 1-wait/inst limit.
    nc = bacc.Bacc("TRN2", target_bir_lowering=False, debug=False,
                   num_devices=N_CORES)
    x_d = nc.declare_dram_parameter("x", [ROWS, D], _F32, isOutput=False)
    if ONCHIP_PE:
        invf_d = nc.declare_dram_parameter("invf", [1, HALF], _F32, isOutput=False)
        sval_d = nc.declare_dram_parameter("sval", [P, PE_BLK], _F32, isOutput=False)
    else:
        pe_d = nc.declare_dram_parameter("pe", [S_SH, D], _F32, isOutput=False)
    out_d = nc.declare_dram_parameter("out", [ROWS, D], _F32, isOutput=True)

    # [p, n, :] = flat row n*128+p. Row r has pe row r mod 512 = (n mod 4)*128+p,
    # so row-block n pairs with pe row-block (n mod 4).
    xv = x_d[:, :].rearrange("(n p) d -> p n d", p=P)     # [128, 32, 1024]
    ov = out_d[:, :].rearrange("(n p) d -> p n d", p=P)

    # tile sizes in row-blocks; shrink the final tiles to cut the tail
    # (last add + last store sit on the critical path after the last load)
    sizes = [K] * (NBLK // K)
    if K >= 2:
        half = K // 2
        sizes = sizes[:-1] + [half] + [1] * (K - half)
    assert sum(sizes) == NBLK

    x_bufs = X_BUFS if X_BUFS else len(sizes)
    with tile.TileContext(nc) as tc:
        with tc.tile_pool(name="pe", bufs=1) as pe_pool, \
             tc.tile_pool(name="x", bufs=x_bufs) as x_pool:
            pe_t = pe_pool.tile([P, PE_BLK, D], _F32)
            if ONCHIP_PE:
                f_t = pe_pool.tile([P, HALF], _F32, name="f_t", tag="f_t")
                nc.sync.dma_start(f_t[:], invf_d[0:1, :].partition_broadcast(P))
                s_t = pe_pool.tile([P, PE_BLK], _F32, name="s_t", tag="s_t")
                nc.sync.dma_start(s_t[:], sval_d[:, :])
                for m in range(PE_BLK):
                    _emit_pe_block(nc, pe_pool, pe_t, f_t, s_t, m)
            else:
                pev = pe_d[:, :].rearrange("(m p) d -> p m d", p=P)  # [128,4,1024]
                nc.sync.dma_start(pe_t[:], pev[:])
            n0 = 0
            for i, sz in enumerate(sizes):
                t = x_pool.tile([P, sz, D], _F32, name="t", tag="t")
                # alternate load issue ring: Sync (HWDGE) / GpSimd (SWDGE)
                ld_eng = nc.sync if (i % 2 == 0 or not ALT_RINGS) else nc.gpsimd
                ld_eng.dma_start(t[:], xv[:, n0:n0 + sz, :])
                r = 0
                while r < sz:
                    m = (n0 + r) % PE_BLK
                    c = min(sz - r, PE_BLK - m)
                    sl = t[:, r:r + c, :]
                    nc.vector.tensor_add(sl, sl, pe_t[:, m:m + c, :])
                    r += c
                nc.scalar.dma_start(ov[:, n0:n0 + sz, :], t[:])
                n0 += sz
    nc.finalize()
    _nc_cache = nc
    return nc


def _inv_freq():
    """inv_freq row [1, D/2], matching the reference's jnp computation
    bit-for-bit when jax is available (jnp.power differs from np.power by
    1 ulp for some j, which the pos multiply amplifies to ~4e-4 in sin)."""
    try:
        import jax.numpy as jnp

        j = jnp.arange(D // 2, dtype=jnp.float32)[None, :]
        return np.asarray(jnp.power(10000.0, -2.0 * j / D), dtype=np.float32)
    except Exception:
        j = np.arange(D // 2, dtype=np.float32)[None, :]
        return np.power(np.float32(10000.0), np.float32(-2.0) * j / np.float32(D))


def _pos_encoding():
    """pe table, replicating reference's fp32 jax computation. Use jax when
    importable so the values match the reference bit-for-bit on the same
    backend; fall back to a float32 numpy pipeline (~1e-7 off per element,
    worst-case ~4e-4 after the pos*inv_freq f32 rounding amplification)."""
    try:
        import jax
        import jax.numpy as jnp

        pos = jnp.arange(S, dtype=jnp.float32)[:, None]
        j = jnp.arange(D // 2, dtype=jnp.float32)[None, :]
        inv_freq = jnp.power(10000.0, -2.0 * j / D)
        angles = pos * inv_freq
        pe = jnp.empty((S, D), dtype=jnp.float32)
        pe = pe.at[:, 0::2].set(jnp.sin(angles))
        pe = pe.at[:, 1::2].set(jnp.cos(angles))
        return np.asarray(pe, dtype=np.float32)
    except Exception:
        pos = np.arange(S, dtype=np.float32)[:, None]
        j = np.arange(D // 2, dtype=np.float32)[None, :]
        inv_freq = np.power(np.float32(10000.0),
                            np.float32(-2.0) * j / np.float32(D))
        angles = pos * inv_freq
        pe = np.empty((S, D), dtype=np.float32)
        pe[:, 0::2] = np.sin(angles)
        pe[:, 1::2] = np.cos(angles)
        return pe


def _run(x, trace=False):
    x = np.ascontiguousarray(np.asarray(x, dtype=np.float32))
    nc = _build_nc()
    in_maps = []
    if ONCHIP_PE:
        invf = np.ascontiguousarray(_inv_freq())
        p_idx = np.arange(P, dtype=np.float32)[:, None]
        m_idx = np.arange(PE_BLK, dtype=np.float32)[None, :]
        for k in range(N_CORES):
            xk = np.ascontiguousarray(
                x[:, k * S_SH:(k + 1) * S_SH, :]).reshape(ROWS, D)
            sval = (k * S_SH + m_idx * P + p_idx).astype(np.float32)
            in_maps.append({"x": xk, "invf": invf,
                            "sval": np.ascontiguousarray(sval)})
    else:
        pe = _pos_encoding()
        for k in range(N_CORES):
            xk = np.ascontiguousarray(
                x[:, k * S_SH:(k + 1) * S_SH, :]).reshape(ROWS, D)
            pek = np.ascontiguousarray(pe[k * S_SH:(k + 1) * S_SH, :])
            in_maps.append({"x": xk, "pe": pek})
    res = run_bass_kernel_spmd(nc, in_maps, list(range(N_CORES)), trace=trace)
    outs = [res.results[k]["out"].reshape(B, S_SH, D) for k in range(N_CORES)]
    full = np.concatenate(outs, axis=1)
    return full, res


def kernel(x):
    # one retry: transient NRT_EXEC_UNIT_UNRECOVERABLE wedges have been
    # observed to clear on a fresh attempt
    try:
        return _run(x, trace=False)[0]
    except Exception:
        import time
        time.sleep(10)
        return _run(x, trace=False)[0]
